# revision 19
# baseline (speedup 1.0000x reference)
"""Trainium2 Bass kernel for nn_Attention_70033736728830 (sparse attention +
linear-attention memory module). Data-parallel over batch: B=128 -> 8 cores x 16.

On-chip layout is feature-major ([feature, token]); weights pre-transposed on
host. f32r for the q/k projection path, bf16 for attention weights, values and
the memory-update path (all accumulation in f32 PSUM).
Returns (out, mem_new, z_new) like the reference.
"""
import os
import numpy as np

B, N, C = 128, 388, 768
H, D = 12, 64
NT, NS = 128, 260          # template/search token counts
CT = 6                     # C/128 feature tiles
NCORES = 8
BL = int(os.environ.get("KBL", "16"))   # batches per core
STAGES = int(os.environ.get("KSTAGES", "7"))
NOBIAS_MM = os.environ.get("KNOBIAS", "0") == "1"   # debug: skip K=1 bias matmuls
SPLIT_DVE = os.environ.get("KSPLIT", "0") == "1"    # debug: per-bank DVE reads
NO_U = os.environ.get("KNOU", "0") == "1"
SUB6 = int(os.environ.get("KSUB6", "4"))
PRED_PLAIN = os.environ.get("KPREDPLAIN", "0") == "1"

SCH = [(0, 128), (128, 128), (256, 4)]                 # search-token chunks (260)
TCH = [(0, 128), (128, 128), (256, 128), (384, 4)]     # all-token chunks (388)
FSP = [(0, 512), (512, 256)]                           # 768-wide free splits

_cache = {}


def _build(bl):
    import concourse.bacc as bacc
    import concourse.mybir as mybir
    import concourse.tile as tile

    F32 = mybir.dt.float32
    F32R = mybir.dt.float32r
    BF16 = mybir.dt.bfloat16
    Alu = mybir.AluOpType
    Act = mybir.ActivationFunctionType

    nc = bacc.Bacc('TRN2', target_bir_lowering=False, debug=False,
                   num_devices=NCORES)

    def par(name, shape, dt, out=False):
        return nc.declare_dram_parameter(name, shape, dt, isOutput=out)

    xT_e = par('xT', [bl, 128, CT, N], F32R)
    memP_e = par('memP', [bl, 128, 384], BF16)
    memPf_e = par('memPf', [bl, 128, 384], F32)
    zBD_e = par('zBD', [bl, 128, CT, 12], BF16)
    zcol_e = par('zcol', [bl, 128, CT], F32)

    qkvw_e = par('qkvw', [128, CT, 2304], F32R)
    memkw_e = par('memkw', [128, CT, C], F32R)
    memvw_e = par('memvw', [128, CT, C], BF16)
    projw_e = par('projw', [128, CT, C], BF16)
    qkvb_e = par('qkvb', [128, 18], F32)
    qkvbvr_e = par('qkvbvr', [1, C], F32R)     # v-part of qkv bias, as a row
    memkb_e = par('memkb', [128, CT], F32)
    memkb1_e = par('memkb1', [128, CT], F32)
    memkbr_e = par('memkbr', [1, C], F32R)
    memvbr_e = par('memvbr', [1, C], BF16)
    projbb_e = par('projbb', [128, C], F32)
    betasF_e = par('betasF', [64, H], F32)
    ind2_e = par('ind2', [2, 128], F32R)
    e2col_e = par('e2col', [128, 4], BF16)
    onesr_e = par('onesr', [1, 128], F32R)
    ebc_e = par('ebc', [12, H * 64], F32R)
    onesb_e = par('onesb', [1, 128], BF16)

    out_e = par('out', [bl, N, C], F32, out=True)
    memnew_e = par('memnew', [bl, 128, 384], F32, out=True)
    znew_e = par('znew', [bl, 128, CT], F32, out=True)

    with tile.TileContext(nc) as tc:
        with tc.tile_pool(name='wpool', bufs=1) as wp, \
             tc.tile_pool(name='dpool', bufs=2) as dp, \
             tc.tile_pool(name='apool', bufs=1) as ap, \
             tc.tile_pool(name='spool', bufs=2) as sp, \
             tc.tile_pool(name='spool1', bufs=1) as sp1, \
             tc.tile_pool(name='epool', bufs=4) as ep, \
             tc.tile_pool(name='pbig', bufs=2, space='PSUM') as pbig, \
             tc.tile_pool(name='pmid', bufs=4, space='PSUM') as pmid, \
             tc.tile_pool(name='pwide', bufs=1, space='PSUM') as pwide:

            qkvw = wp.tile([128, CT, 2304], F32R, tag='qkvw')
            memkw = wp.tile([128, CT, C], F32R, tag='memkw')
            memvw = wp.tile([128, CT, C], BF16, tag='memvw')
            projw = wp.tile([128, CT, C], BF16, tag='projw')
            qkvb = wp.tile([128, 18], F32, tag='qkvb')
            qkvbvr = wp.tile([1, C], F32R, tag='qkvbvr')
            memkb = wp.tile([128, CT], F32, tag='memkb')
            memkb1 = wp.tile([128, CT], F32, tag='memkb1')
            memkbr = wp.tile([1, C], F32R, tag='memkbr')
            memvbr = wp.tile([1, C], BF16, tag='memvbr')
            projbb = wp.tile([128, C], F32, tag='projbb')
            betasF = wp.tile([64, H], F32, tag='betasF')
            bbF = wp.tile([64, H], F32, tag='bbF')
            bb1mF = wp.tile([64, H], F32, tag='bb1mF')
            ind2 = wp.tile([2, 128], F32R, tag='ind2')
            e2col = wp.tile([128, 4], BF16, tag='e2col')
            onesr = wp.tile([1, 128], F32R, tag='onesr')
            ebc = wp.tile([12, H * 64], F32R, tag='ebc')
            onesb = wp.tile([1, 128], BF16, tag='onesb')

            for t_sb, t_e in [(qkvw, qkvw_e), (memkw, memkw_e), (memvw, memvw_e),
                              (projw, projw_e), (qkvb, qkvb_e), (qkvbvr, qkvbvr_e),
                              (memkb, memkb_e), (memkb1, memkb1_e),
                              (memkbr, memkbr_e), (memvbr, memvbr_e),
                              (projbb, projbb_e), (betasF, betasF_e),
                              (ind2, ind2_e), (e2col, e2col_e),
                              (onesr, onesr_e), (onesb, onesb_e),
                              (ebc, ebc_e)]:
                nc.sync.dma_start(t_sb[:], t_e[:])

            nc.scalar.activation(bbF[:], betasF[:], Act.Sigmoid)
            nc.vector.tensor_scalar(out=bb1mF[:], in0=bbF[:], scalar1=-1.0,
                                    scalar2=1.0, op0=Alu.mult, op1=Alu.add)

            for b in range(bl):
                xT = dp.tile([128, CT, N], F32R, tag='xT')
                memP = dp.tile([128, 384], BF16, tag='memP')
                memPf = dp.tile([128, 384], F32, tag='memPf')
                zBD = dp.tile([128, CT, 12], BF16, tag='zBD')
                zcol = dp.tile([128, CT], F32, tag='zcol')
                nc.sync.dma_start(xT[:], xT_e[b])
                nc.sync.dma_start(memP[:], memP_e[b])
                nc.sync.dma_start(memPf[:], memPf_e[b])
                nc.sync.dma_start(zBD[:], zBD_e[b])
                nc.sync.dma_start(zcol[:], zcol_e[b])

                qT = ap.tile([128, CT, N], F32R, tag='qT')
                kT = ap.tile([128, CT, N], F32R, tag='kT')
                vT = ap.tile([128, CT, N], BF16, tag='vT')
                sigq = ap.tile([128, CT, NS], BF16, tag='sigq')
                sigk = ap.tile([128, CT, NS], BF16, tag='sigk')
                sigktok = ap.tile([128, 3, C], BF16, tag='sigktok')
                vtok = ap.tile([128, 4, C], BF16, tag='vtok')
                diff = ap.tile([128, 3, C], BF16, tag='diff')
                xcat = ap.tile([128, CT, N], BF16, tag='xcat')
                zz = ap.tile([128, CT], F32, tag='zz')
                nrk = ap.tile([128, 3, 12], F32, tag='nrk')

                # ---- 1. qkv projection, feature-major out ----
                for m in range(18):
                    ps = pbig.tile([128, N], F32, tag='pb')
                    for k in range(CT):
                        nc.tensor.matmul(ps[:, :], qkvw[:, k, m * 128:(m + 1) * 128],
                                         xT[:, k, :], start=(k == 0), stop=(k == CT - 1))
                    if m < 6:
                        nc.scalar.activation(qT[:, m, :], ps[:, :], Act.Identity,
                                             bias=qkvb[:, m:m + 1])
                    elif m < 12:
                        nc.scalar.activation(kT[:, m - 6, :], ps[:, :], Act.Identity,
                                             bias=qkvb[:, m:m + 1])
                    else:
                        nc.vector.tensor_scalar(out=vT[:, m - 12, :], in0=ps[:, :],
                                                scalar1=qkvb[:, m:m + 1], scalar2=None,
                                                op0=Alu.add)

                if STAGES < 2:
                    continue
                # ---- 2. v token-major via flipped matmul from x ----
                for c, (off, cl) in enumerate(TCH):
                    pvA = pwide.tile([128, 512], F32, tag='pwA')
                    pvB = pwide.tile([128, 256], F32, tag='pwB')
                    pvs0 = {0: pvA, 512: pvB}
                    for fo, fl in FSP:
                        pv = pvs0[fo]
                        for k in range(CT):
                            nc.tensor.matmul(pv[0:cl, 0:fl],
                                             xT[:, k, off:off + cl],
                                             qkvw[:, k, 1536 + fo:1536 + fo + fl],
                                             start=(k == 0), stop=False)
                        nc.tensor.matmul(pv[0:cl, 0:fl],
                                         onesr[0:1, 0:cl], qkvbvr[0:1, fo:fo + fl],
                                         start=False, stop=True)
                        nc.vector.tensor_copy(vtok[0:cl, c, fo:fo + fl], pv[0:cl, 0:fl])

                if STAGES < 3:
                    continue
                # ---- 3. sigma_q = elu(q_search)+1, feature-major, bf16 ----
                for t in range(CT):
                    em = sp.tile([128, NS], F32, tag='em260')
                    nc.vector.tensor_scalar_min(em[:, :], qT[:, t, NT:N], 0.0)
                    nc.scalar.activation(em[:, :], em[:, :], Act.Exp)
                    nc.vector.scalar_tensor_tensor(out=sigq[:, t, :], in0=qT[:, t, NT:N],
                                                   scalar=1.0, in1=em[:, :],
                                                   op0=Alu.add, op1=Alu.max)

                # ---- 4. den_m = sigma_q @ z (per head), then recip rows ----
                dm = pmid.tile([12, NS], F32, tag='pm')
                for t in range(CT):
                    nc.tensor.matmul(dm[0:12, :], zBD[:, t, :], sigq[:, t, :],
                                     start=(t == 0), stop=(t == CT - 1))
                rdf = sp1.tile([12, NS], F32, tag='rdf')
                nc.vector.reciprocal(rdf[:, :], dm[0:12, :])
                rdfr = ap.tile([12, NS], F32R, tag='rdfr')
                nc.vector.tensor_copy(rdfr[:, :], rdf[:, :])

                if STAGES < 4:
                    continue
                # ---- 5. k_s_mem feature-major + elu -> sigk (+zz accum) ----
                for t in range(CT):
                    pk = pbig.tile([128, N], F32, tag='pb')
                    for k in range(CT):
                        nc.tensor.matmul(pk[:, 0:NS], memkw[:, k, t * 128:(t + 1) * 128],
                                         kT[:, k, NT:N], start=(k == 0), stop=(k == CT - 1))
                    em = sp.tile([128, NS], F32, tag='em260')
                    nc.vector.tensor_scalar(out=em[:, :], in0=pk[:, 0:NS],
                                            scalar1=memkb[:, t:t + 1], scalar2=0.0,
                                            op0=Alu.add, op1=Alu.min)
                    nc.scalar.activation(em[:, :], em[:, :], Act.Exp)
                    nc.vector.scalar_tensor_tensor(out=sigk[:, t, :], in0=pk[:, 0:NS],
                                                   scalar=memkb1[:, t:t + 1], in1=em[:, :],
                                                   op0=Alu.add, op1=Alu.max,
                                                   accum_out=zz[:, t:t + 1])

                # ---- 6. z_new ----
                znew = sp.tile([128, CT], F32, tag='znew')
                nc.vector.tensor_tensor(out=znew[:, :], in0=zcol[:, :], in1=zz[:, :],
                                        op=Alu.add)
                nc.sync.dma_start(znew_e[b], znew[:, :])

                if STAGES < 5:
                    continue
                # ---- 7. attention, per head ----
                for h in range(H):
                    t = h // 2
                    half = (h % 2) * 64
                    exps = {}
                    cs = pmid.tile([1, N], F32, tag='pm')
                    for ki, (koff, kl) in enumerate(TCH):
                        qoff, ql = (0, N) if ki == 0 else (NT, NS)
                        ss = pbig.tile([128, N], F32, tag='pb')
                        nc.tensor.matmul(ss[0:kl, 0:ql],
                                         kT[half:half + 64, t, koff:koff + kl],
                                         qT[half:half + 64, t, qoff:qoff + ql],
                                         start=True, stop=True)
                        ex = ep.tile([128, N], BF16, tag='exp')
                        nc.scalar.activation(ex[0:kl, 0:ql], ss[0:kl, 0:ql],
                                             Act.Exp, scale=0.125)
                        exps[ki] = ex
                        nc.tensor.matmul(cs[0:1, qoff:qoff + ql],
                                         e2col[0:kl, 0:1], ex[0:kl, 0:ql],
                                         start=(ki == 0), stop=(ki == 3))
                    xc = pmid.tile([64, N], F32, tag='pm')
                    for ki, (koff, kl) in enumerate(TCH):
                        qoff, ql = (0, N) if ki == 0 else (NT, NS)
                        nc.tensor.matmul(xc[0:64, qoff:qoff + ql],
                                         vtok[0:kl, ki, h * 64:(h + 1) * 64],
                                         exps[ki][0:kl, 0:ql],
                                         start=(ki == 0), stop=(ki == 3))
                    an = pmid.tile([64, NS], F32, tag='pm')
                    nc.tensor.matmul(an[0:64, :],
                                     memP[half:half + 64, t * 64:(t + 1) * 64],
                                     sigq[half:half + 64, t, :],
                                     start=True, stop=True)

                    rrf = sp1.tile([1, N], F32, tag='rrf')
                    nc.vector.reciprocal(rrf[0:1, :], cs[0:1, :])
                    rrow = sp1.tile([1, N], F32R, tag='rrow')
                    nc.vector.tensor_copy(rrow[0:1, :], rrf[0:1, :])
                    bcs_p = pmid.tile([64, N], F32, tag='pm')
                    nc.tensor.matmul(bcs_p[0:64, :], onesr[0:1, 0:64], rrow[0:1, :],
                                     start=True, stop=True)
                    bcs = sp.tile([64, N], F32, tag='bcs')
                    nc.scalar.activation(bcs[0:64, :], bcs_p[0:64, :], Act.Copy)
                    bcm_p = pmid.tile([64, NS], F32, tag='pm')
                    nc.tensor.matmul(bcm_p[0:64, :], ebc[0:12, h * 64:(h + 1) * 64],
                                     rdfr[0:12, :], start=True, stop=True)
                    bcm = sp.tile([64, NS], F32, tag='bcm')
                    nc.scalar.activation(bcm[0:64, :], bcm_p[0:64, :], Act.Copy)

                    nc.vector.tensor_tensor(out=xcat[half:half + 64, t, 0:NT],
                                            in0=xc[0:64, 0:NT],
                                            in1=bcs[0:64, 0:NT], op=Alu.mult)
                    ta = sp.tile([64, NS], F32, tag='ta')
                    tb = sp.tile([64, NS], F32, tag='tb')
                    nc.vector.scalar_tensor_tensor(out=ta[:, :], in0=an[0:64, :],
                                                   scalar=bbF[:, h:h + 1],
                                                   in1=bcm[0:64, :],
                                                   op0=Alu.mult, op1=Alu.mult)
                    nc.vector.scalar_tensor_tensor(out=tb[:, :],
                                                   in0=xc[0:64, NT:N],
                                                   scalar=bb1mF[:, h:h + 1],
                                                   in1=bcs[0:64, NT:N],
                                                   op0=Alu.mult, op1=Alu.mult)
                    nc.vector.tensor_tensor(out=xcat[half:half + 64, t, NT:N],
                                            in0=ta[:, :], in1=tb[:, :], op=Alu.add)

                if STAGES < 6:
                    continue
                # ---- 8. memory update ----
                if SUB6 >= 4:
                    U1 = pmid.tile([64, 384], F32, tag='pm')
                    U2 = pmid.tile([64, 384], F32, tag='pm')
                for c, (off, cl) in enumerate(SCH):
                    # k_s_mem token-major + elu
                    pkA = pwide.tile([128, 512], F32, tag='pwA')
                    pkB = pwide.tile([128, 256], F32, tag='pwB')
                    pks = {0: pkA, 512: pkB}
                    for fo, fl in FSP:
                        pk = pks[fo]
                        for k in range(CT):
                            nc.tensor.matmul(pk[0:cl, 0:fl],
                                             kT[:, k, NT + off:NT + off + cl],
                                             memkw[:, k, fo:fo + fl],
                                             start=(k == 0),
                                             stop=(NOBIAS_MM and k == CT - 1))
                        if not NOBIAS_MM:
                            nc.tensor.matmul(pk[0:cl, 0:fl], onesr[0:1, 0:cl],
                                             memkbr[0:1, fo:fo + fl], start=False, stop=True)
                    em8 = sp1.tile([128, C], F32, tag='em768')
                    for fo, fl in FSP:
                        pk = pks[fo]
                        nc.vector.tensor_scalar_min(em8[0:cl, fo:fo + fl], pk[0:cl, 0:fl], 0.0)
                        nc.scalar.activation(em8[0:cl, fo:fo + fl], em8[0:cl, fo:fo + fl], Act.Exp)
                        nc.vector.scalar_tensor_tensor(out=sigktok[0:cl, c, fo:fo + fl],
                                                       in0=pk[0:cl, 0:fl],
                                                       scalar=1.0, in1=em8[0:cl, fo:fo + fl],
                                                       op0=Alu.add, op1=Alu.max)

                    # den_k -> recip
                    if SUB6 < 2:
                        continue
                    dk = pmid.tile([128, 12], F32, tag='pm')
                    for t in range(CT):
                        nc.tensor.matmul(dk[0:cl, :], sigk[:, t, off:off + cl],
                                         zBD[:, t, :], start=(t == 0), stop=(t == CT - 1))
                    nc.vector.reciprocal(nrk[0:cl, c, :], dk[0:cl, :])

                    # v_s_mem token-major (psum only)
                    pvA = pwide.tile([128, 512], F32, tag='pwA')
                    pvB = pwide.tile([128, 256], F32, tag='pwB')
                    pvs = {0: pvA, 512: pvB}
                    for fo, fl in FSP:
                        pv = pvs[fo]
                        for k in range(CT):
                            nc.tensor.matmul(pv[0:cl, 0:fl],
                                             vT[:, k, NT + off:NT + off + cl],
                                             memvw[:, k, fo:fo + fl],
                                             start=(k == 0),
                                             stop=(NOBIAS_MM and k == CT - 1))
                        if not NOBIAS_MM:
                            nc.tensor.matmul(pv[0:cl, 0:fl], onesb[0:1, 0:cl],
                                             memvbr[0:1, fo:fo + fl], start=False, stop=True)

                    # pred (pairs share one psum), predn, diff
                    if SUB6 < 3:
                        nc.vector.tensor_copy(diff[0:cl, c, 0:512], pvs[0][0:cl, 0:512])
                        nc.vector.tensor_copy(diff[0:cl, c, 512:768], pvs[512][0:cl, 0:256])
                        continue
                    predn = sp1.tile([128, C], BF16, tag='predn')
                    for p2 in range(6):
                        pph = {}
                        for hi, h in enumerate((2 * p2, 2 * p2 + 1)):
                            half = (h % 2) * 64
                            pp = pmid.tile([128, 64], F32, tag='pm')
                            nc.tensor.matmul(pp[0:cl, 0:64],
                                             sigk[half:half + 64, p2, off:off + cl],
                                             memP[half:half + 64, p2 * 64:(p2 + 1) * 64],
                                             start=True, stop=True)
                            pph[hi] = pp
                        for hi, h in enumerate((2 * p2, 2 * p2 + 1)):
                            nc.vector.tensor_scalar(out=predn[0:cl, h * 64:(h + 1) * 64],
                                                    in0=pph[hi][0:cl, 0:64],
                                                    scalar1=nrk[0:cl, c, h:h + 1],
                                                    scalar2=None, op0=Alu.mult)
                    for fo, fl in FSP:
                        nc.vector.scalar_tensor_tensor(out=diff[0:cl, c, fo:fo + fl],
                                                       in0=pvs[fo][0:cl, 0:fl],
                                                       scalar=0.0, in1=predn[0:cl, fo:fo + fl],
                                                       op0=Alu.add, op1=Alu.subtract)
                for h in range(H if (not NO_U and SUB6 >= 4) else 0):
                    U = U1 if h < 6 else U2
                    for c, (off, cl) in enumerate(SCH):
                        nc.tensor.matmul(U[0:64, (h % 6) * 64:(h % 6 + 1) * 64],
                                         sigktok[0:cl, c, h * 64:(h + 1) * 64],
                                         diff[0:cl, c, h * 64:(h + 1) * 64],
                                         start=(h % 6 == 0 and c == 0),
                                         stop=(h % 6 == 5 and c == 2))

                memnew = sp1.tile([128, 384], F32, tag='memnew')
                if SUB6 >= 4:
                    for h in range(H):
                        half = (h % 2) * 64
                        U = U1 if h < 6 else U2
                        nc.vector.tensor_tensor(
                            out=memnew[half:half + 64, (h // 2) * 64:(h // 2 + 1) * 64],
                            in0=memPf[half:half + 64, (h // 2) * 64:(h // 2 + 1) * 64],
                            in1=U[0:64, (h % 6) * 64:(h % 6 + 1) * 64], op=Alu.add)
                else:
                    nc.vector.tensor_copy(memnew[:, :], memPf[:, :])
                nc.sync.dma_start(memnew_e[b], memnew[:, :])

                if STAGES < 7:
                    continue
                # ---- 9. output projection (token-major) ----
                for c, (off, cl) in enumerate(TCH):
                    poA = pwide.tile([128, 512], F32, tag='pwA')
                    poB = pwide.tile([128, 256], F32, tag='pwB')
                    pos = {0: poA, 512: poB}
                    oc = sp1.tile([128, C], F32, tag='oc')
                    for fo, fl in FSP:
                        po = pos[fo]
                        for k in range(CT):
                            nc.tensor.matmul(po[0:cl, 0:fl],
                                             xcat[:, k, off:off + cl],
                                             projw[:, k, fo:fo + fl],
                                             start=(k == 0), stop=(k == CT - 1))
                        nc.vector.scalar_tensor_tensor(out=oc[0:cl, fo:fo + fl],
                                                       in0=po[0:cl, 0:fl],
                                                       scalar=0.0, in1=projbb[0:cl, fo:fo + fl],
                                                       op0=Alu.add, op1=Alu.add)
                    nc.sync.dma_start(out_e[b, off:off + cl, :], oc[0:cl, :])

    nc.finalize()
    return nc


def _to_bf16(a):
    import ml_dtypes
    return np.asarray(a, np.float32).astype(ml_dtypes.bfloat16)


def _prep(inputs, bl):
    x = np.asarray(inputs['x'], np.float32)
    mem = np.asarray(inputs['mem'], np.float32)
    z = np.asarray(inputs['z'], np.float32)
    qkv_w = np.asarray(inputs['qkv_w'], np.float32)
    qkv_b = np.asarray(inputs['qkv_b'], np.float32)
    proj_w = np.asarray(inputs['proj_w'], np.float32)
    proj_b = np.asarray(inputs['proj_b'], np.float32)
    memk_w = np.asarray(inputs['memk_w'], np.float32)
    memk_b = np.asarray(inputs['memk_b'], np.float32)
    memv_w = np.asarray(inputs['memv_w'], np.float32)
    memv_b = np.asarray(inputs['memv_b'], np.float32)
    betas = np.asarray(inputs['betas'], np.float32)
    nb = bl * NCORES

    # activations / state, per batch
    xt = np.swapaxes(x[:nb], 1, 2)                                   # [nb,768,388]
    xTr = np.ascontiguousarray(
        xt.reshape(nb, CT, 128, N).transpose(0, 2, 1, 3))
    m2 = mem[:nb].reshape(nb, CT, 2, 64, 64).transpose(0, 2, 3, 1, 4)
    memP = np.ascontiguousarray(m2.reshape(nb, 128, 384))
    zf = z[:nb].reshape(nb, H, 64)
    zBD = np.zeros((nb, CT, 128, H), np.float32)
    for t in range(CT):
        for hf in range(2):
            h = 2 * t + hf
            zBD[:, t, hf * 64:(hf + 1) * 64, h] = zf[:, h, :]
    zBD = np.ascontiguousarray(zBD.transpose(0, 2, 1, 3))
    zcol = np.ascontiguousarray(
        zf.reshape(nb, C).reshape(nb, CT, 128).transpose(0, 2, 1))  # [nb,128,6]

    # weights
    def tile_w(wT, width):   # [768, width] -> [128, 6, width]
        return np.ascontiguousarray(wT.reshape(CT, 128, width).transpose(1, 0, 2))

    qkvw = tile_w(np.ascontiguousarray(qkv_w.T), 2304)
    memkw = tile_w(np.ascontiguousarray(memk_w.T), C)
    memvw = tile_w(np.ascontiguousarray(memv_w.T), C)
    projw = tile_w(np.ascontiguousarray(proj_w.T), C)
    qkvbP = np.ascontiguousarray(qkv_b.reshape(18, 128).T)
    memkbP = np.ascontiguousarray(memk_b.reshape(CT, 128).T)
    ind2 = np.zeros((2, 128), np.float32)
    ind2[0, 0:64] = 1.0
    ind2[1, 64:128] = 1.0
    e2col = np.zeros((128, 4), np.float32)
    e2col[:, 0] = 1.0
    e2col[:, 3] = 1.0
    ones128 = np.ones((1, 128), np.float32)
    ebc = np.zeros((12, H * 64), np.float32)
    for h in range(H):
        ebc[h, h * 64:(h + 1) * 64] = 1.0

    shared = dict(
        qkvw=qkvw, memkw=memkw, memvw=_to_bf16(memvw), projw=_to_bf16(projw),
        qkvb=qkvbP, qkvbvr=qkv_b[1536:2304].reshape(1, C).copy(),
        memkb=memkbP, memkb1=memkbP + 1.0,
        memkbr=memk_b.reshape(1, C).copy(), memvbr=_to_bf16(memv_b.reshape(1, C)),
        projbb=np.ascontiguousarray(np.tile(proj_b, (128, 1))),
        betasF=np.ascontiguousarray(betas[0, :, 0, :].T),
        ind2=ind2, e2col=_to_bf16(e2col), onesr=ones128, onesb=_to_bf16(ones128),
        ebc=ebc,
    )
    in_maps = []
    for i in range(NCORES):
        s = slice(i * bl, (i + 1) * bl)
        m = dict(shared)
        m.update(xT=xTr[s], memP=_to_bf16(memP[s]), memPf=memP[s],
                 zBD=_to_bf16(zBD[s]), zcol=zcol[s])
        in_maps.append(m)
    return in_maps


def _postprocess(results, bl):
    nb = bl * len(results)
    out = np.concatenate([r['out'] for r in results], axis=0)
    mP = np.concatenate([r['memnew'] for r in results], axis=0)
    mem_new = np.ascontiguousarray(
        mP.reshape(nb, 2, 64, CT, 64).transpose(0, 3, 1, 2, 4)).reshape(nb, H, 64, 64)
    zn = np.concatenate([r['znew'] for r in results], axis=0)
    z_new = np.ascontiguousarray(zn.transpose(0, 2, 1)).reshape(nb, H, 64, 1)
    return out, mem_new, z_new


def kernel(**inputs):
    from concourse.bass_utils import run_bass_kernel_spmd
    bl = BL
    if 'nc' not in _cache:
        _cache['nc'] = _build(bl)
    nc = _cache['nc']
    in_maps = _prep(inputs, bl)
    res = run_bass_kernel_spmd(nc, in_maps, core_ids=list(range(NCORES)))
    return _postprocess(res.results, bl)
